# revision 20
# baseline (speedup 1.0000x reference)
"""Trainium2 Bass kernel for per-pixel dynamic-weight 3x3 aggregation.

Computation (per sample):
    out[c, h, w] = sum_{kh,kw} xpad[c, h+kh, w+kw] * weight[c % WC, kh*3+kw, h, w]
with reflect padding (pad=1) of x.

Sharding: data-parallel over batch N=8 -> one sample per NeuronCore (8 cores).

v3 design:
  - f16 end-to-end: host casts x/w to f16 and casts the f16 output back.
  - Host pre-packs x and w into per-(chunk, partition)-contiguous layouts:
    每 partition's whole chunk tile is one contiguous DRAM run (x: 20.5KB,
    w: 18.4KB), so a chunk loads with ONE DMA of 128 big descriptors. The
    DMA path is descriptor-rate-bound (~10ns/desc), so this cuts the load
    path from ~9900 descriptors (99us) to ~1000 (~53us, now bus-bound).
    The x pack also materializes the row halo + row-reflect; the w pack
    applies the per-tap column shifts (taps read column-aligned), the
    reflect-column folds, and zeroes the shift-garbage slots. No device
    memsets or reflect DMAs remain.
  - The +-1 tap column shifts are undone at accumulation: PE identity-matmul
    windows (out[f + 1-kw] += p[f]) per PSUM bank.
  - Reflect columns fold into weights (exact): out[.,0]'s reflect term
    x[.,1]*w_k0[.,0] and its kw=2 term share the x factor -> host adds
    w_k0[:,0] into w_k2[:,0] (symmetric at col 127).
  - DVE does only the 9 tap products: 3 mega tensor_mul per phase (one per
    kw, all kh at once, 6144 els at 2x) -> ~169us busy = the bottleneck.
  - PE tap-sum ~123us, ACT evac f32->f16 ~30us, DMA ~76us: all hidden.
  - Output stored f16 to a packed layout, host unpacks + casts to f32.

Partition mapping: p = q*32 + wc (q = row-quarter of the chunk, wc = weight
channel). Free dims = (g, row, col), channel c = g*32 + wc.
"""

import numpy as np

import concourse.tile as tile
from concourse import bacc, mybir
from concourse.ap import AP
from concourse.bass_utils import run_bass_kernel_spmd

# Problem constants (hardcoded per contract).
N, C, H, W = 8, 256, 128, 128
WC, KK = 32, 9
G = C // WC  # 8 channel groups share one weight channel
NCORES = 8

R = 32            # rows per chunk
NCHUNK = H // R   # 4
Q = R // 4        # 8 rows handled per partition (one quarter of a chunk)
XROWS = Q + 2     # rows in the x tiles (1-row halo on each side)

FP32 = mybir.dt.float32
F16 = mybir.dt.float16

HW_ = H * W
QW = Q * W
XSZ = G * XROWS * W      # 10240 x elements per partition per chunk
WSZ = KK * QW            # 9216 w elements per partition per chunk
OSZ = 2 * QW             # 2048 out elements per partition per phase

_compiled = None


def _dram_ap(t, offset, dims):
    """AP over a DRAM tensor with explicit [stride, count] dims (elements)."""
    return AP(tensor=t.ap().tensor, offset=int(offset), ap=[[int(s), int(c)] for s, c in dims])


def _sb_ap(base, offset, dims):
    """AP over an SBUF tile: keep its partition dim, custom free dims."""
    return AP(
        tensor=base.tensor,
        offset=base.offset + int(offset),
        ap=[list(base.ap[0])] + [[int(s), int(c)] for s, c in dims],
    )


def build(
    reps: int = 1,
    do_dma: bool = True,
    do_compute: bool = True,
    do_store: bool | None = None,
    do_pe: bool | None = None,
):
    do_load = do_dma
    do_store = (do_dma if do_store is None else do_store)
    do_dve = do_compute
    do_pe = (do_compute if do_pe is None else do_pe) and do_dve
    do_store = do_store and do_pe  # stores read osb, written by evac
    nc = bacc.Bacc("TRN2", target_bir_lowering=False, debug=False, num_devices=1)

    x_t = nc.dram_tensor("xp", [NCHUNK, 128, XSZ], F16, kind="ExternalInput")
    w_t = nc.dram_tensor("wp", [NCHUNK, 128, WSZ], F16, kind="ExternalInput")
    id_t = nc.dram_tensor("ident", [128, 128], F16, kind="ExternalInput")
    o_t = nc.dram_tensor("outp", [NCHUNK, 4, 128, OSZ], F16, kind="ExternalOutput")

    with tile.TileContext(nc) as tc:
        with (
            tc.tile_pool(name="const", bufs=1) as const_pool,
            tc.tile_pool(name="xe", bufs=2) as xe_pool,
            tc.tile_pool(name="wp", bufs=2) as w_pool,
            tc.tile_pool(name="prod", bufs=3) as prod_pool,
            tc.tile_pool(name="osb", bufs=3) as out_pool,
            tc.tile_pool(name="ps", bufs=2, space="PSUM") as psum_pool,
        ):
            ident = const_pool.tile([128, 128], F16)
            nc.sync.dma_start(ident[:], id_t.ap())

            def load_chunk(ch):
                xe = xe_pool.tile([128, G, XROWS, W], F16, tag="xe")
                wt = w_pool.tile([128, KK, Q, W], F16, tag="wt")
                if do_load:
                    src = _dram_ap(x_t, ch * 128 * XSZ, [(XSZ, 128), (1, XSZ)])
                    nc.sync.dma_start(
                        xe[:].rearrange("p a b c -> p (a b c)"), src
                    )
                    src = _dram_ap(w_t, ch * 128 * WSZ, [(WSZ, 128), (1, WSZ)])
                    nc.sync.dma_start(
                        wt[:].rearrange("p a b c -> p (a b c)"), src
                    )
                return xe, wt

            def run_chunk(ch, tiles):
                xe, wt = tiles
                for ph in range(4):  # g-pair phases: g in {2ph, 2ph+1}
                    pkw0 = prod_pool.tile([128, 3, 2, Q, W], F16, tag="pkw0")
                    pkw1 = prod_pool.tile([128, 3, 2, Q, W], F16, tag="pkw1")
                    pkw2 = prod_pool.tile([128, 3, 2, Q, W], F16, tag="pkw2")
                    pkw = [pkw0, pkw1, pkw2]
                    if do_dve:
                        # 3 mega multiplies: all kh for one kw in one DVE op.
                        # kw=1 first: it's the start matmul of every PSUM bank.
                        for kw in (1, 0, 2):
                            xin = _sb_ap(
                                xe[:],
                                2 * ph * XROWS * W,
                                [(W, 3), (XROWS * W, 2), (W, Q), (1, W)],
                            )
                            win = _sb_ap(
                                wt[:],
                                kw * QW,
                                [(3 * QW, 3), (0, 2), (W, Q), (1, W)],
                            )
                            nc.vector.tensor_mul(pkw[kw][:], xin, win)

                    pst = psum_pool.tile([128, 2048], FP32)
                    if do_pe:
                        # PE tap-sum: per PSUM bank, windowed identity matmuls
                        # out[f + (1-kw)] += p[f]. kw-major order so PE can
                        # start on pkw1 before the kw0/kw2 megas finish.
                        for kw, khi in (
                            (1, 0), (1, 1), (1, 2),
                            (0, 0), (0, 1), (0, 2),
                            (2, 0), (2, 1), (2, 2),
                        ):
                            s = 1 - kw
                            pflat = pkw[kw][:, khi].rearrange(
                                "p g r c -> p (g r c)"
                            )
                            for b in range(4):
                                j0 = max(512 * b, s) if s > 0 else 512 * b
                                j1 = min(512 * b + 512, 2048 + min(s, 0))
                                nc.tensor.matmul(
                                    pst[:, j0:j1],
                                    ident[:],
                                    pflat[:, j0 - s : j1 - s],
                                    start=(kw, khi) == (1, 0),
                                    stop=(kw, khi) == (2, 2),
                                )
                    osb = out_pool.tile([128, 2048], F16)
                    if do_pe:
                        nc.scalar.copy(osb[:], pst[:])
                    if do_store:
                        dst = _dram_ap(
                            o_t,
                            (ch * 4 + ph) * 128 * OSZ,
                            [(OSZ, 128), (1, OSZ)],
                        )
                        nc.scalar.dma_start(dst, osb[:])

            def emit_body():
                # software-pipelined: prefetch chunk ch+1 before computing ch
                tiles = load_chunk(0)
                for ch in range(NCHUNK):
                    nxt = load_chunk(ch + 1) if ch + 1 < NCHUNK else None
                    run_chunk(ch, tiles)
                    tiles = nxt

            if reps == 1:
                emit_body()
            else:  # timing builds: repeat the whole kernel on-device
                with tc.For_i(
                    0, reps, 1,
                    hint_engines=(mybir.EngineType.PE, mybir.EngineType.DVE),
                ):
                    emit_body()

    nc.compile()
    return nc


def _get_compiled():
    global _compiled
    if _compiled is None:
        _compiled = build()
    return _compiled


def make_core_inputs(x_i: np.ndarray, w_i: np.ndarray) -> dict:
    """Host-side packing for one sample (layout + exact weight preprocessing).

    x pack: xp[ch, p=(q,wc), (g, t, c)] = x[g*32+wc, ch*32+q*8-1+t, c] with
    row-reflect at the image edges -- each partition's chunk tile is one
    contiguous run.

    w pack: wp[ch, p=(q,wc), (k, r, c)] = w'[wc, k, flat (rs+r)*W + c + 1-kw]
    where rs = ch*32+q*8 and w' has the reflect-column folds applied
    (w_k2[:,0] += w_k0[:,0]; w_k0[:,127] += w_k2[:,127]) and the shift-garbage
    source columns zeroed (w_k0[:,:,0] = 0, w_k2[:,:,127] = 0, and the two
    plane-edge slots read 0 via padding).
    """
    xv = np.asarray(x_i, dtype=np.float16).reshape(G, WC, H, W)

    w32 = np.array(w_i, dtype=np.float32)  # [WC, KK, H, W]
    w32[:, 2::3, :, 0] += w32[:, 0::3, :, 0]
    w32[:, 0::3, :, 127] += w32[:, 2::3, :, 127]
    w32[:, 0::3, :, 0] = 0.0
    w32[:, 2::3, :, 127] = 0.0
    wflat = np.zeros((WC, KK, HW_ + 2), dtype=np.float16)
    wflat[:, :, 1 : 1 + HW_] = w32.reshape(WC, KK, HW_).astype(np.float16)

    xp = np.empty((NCHUNK, 4, WC, G, XROWS, W), dtype=np.float16)
    wp = np.empty((NCHUNK, 4, WC, KK, Q, W), dtype=np.float16)
    for ch in range(NCHUNK):
        for q in range(4):
            rs = ch * R + q * Q
            rows = np.arange(rs - 1, rs + Q + 1)
            rows[rows == -1] = 1
            rows[rows == H] = H - 2
            xp[ch, q] = xv[:, :, rows, :].transpose(1, 0, 2, 3)
            for k in range(KK):
                off = rs * W + (1 - k % 3) + 1
                wp[ch, q, :, k] = wflat[:, k, off : off + QW].reshape(WC, Q, W)
    return {
        "xp": xp.reshape(NCHUNK, 128, XSZ),
        "wp": wp.reshape(NCHUNK, 128, WSZ),
        "ident": np.eye(128, dtype=np.float16),
    }


def unpack_output(outp: np.ndarray) -> np.ndarray:
    """outp [NCHUNK, 4ph, 128p, OSZ] f16 -> out [C, H, W] f32."""
    o = outp.reshape(NCHUNK, 4, 4, WC, 2, Q, W).astype(np.float32)
    # indices: [ch, ph, q, wc, g', r, c] -> channel (2ph+g')*32+wc, row ch*32+q*8+r
    o = o.transpose(1, 4, 3, 0, 2, 5, 6)  # [ph, g', wc, ch, q, r, c]
    return np.ascontiguousarray(o.reshape(C, H, W))


def kernel(x: np.ndarray, weight: np.ndarray) -> np.ndarray:
    nc = _get_compiled()
    in_maps = [make_core_inputs(x[i], weight[i]) for i in range(NCORES)]
    res = run_bass_kernel_spmd(nc, in_maps, core_ids=list(range(NCORES)))
    return np.stack(
        [unpack_output(res.results[i]["outp"]) for i in range(NCORES)], axis=0
    )


# revision 23
# speedup vs baseline: 1.1610x; 1.1610x over previous
"""Trainium2 Bass kernel for per-pixel dynamic-weight 3x3 aggregation.

Computation (per sample):
    out[c, h, w] = sum_{kh,kw} xpad[c, h+kh, w+kw] * weight[c % WC, kh*3+kw, h, w]
with reflect padding (pad=1) of x.

Sharding: data-parallel over batch N=8 -> one sample per NeuronCore (8 cores).

v3 design:
  - f16 end-to-end: host casts x/w to f16 and casts the f16 output back.
  - Host pre-packs x and w into per-(chunk, partition)-contiguous layouts:
    每 partition's whole chunk tile is one contiguous DRAM run (x: 20.5KB,
    w: 18.4KB), so a chunk loads with ONE DMA of 128 big descriptors. The
    DMA path is descriptor-rate-bound (~10ns/desc), so this cuts the load
    path from ~9900 descriptors (99us) to ~1000 (~53us, now bus-bound).
    The x pack also materializes the row halo + row-reflect; the w pack
    applies the per-tap column shifts (taps read column-aligned), the
    reflect-column folds, and zeroes the shift-garbage slots. No device
    memsets or reflect DMAs remain.
  - The +-1 tap column shifts are undone at accumulation: PE identity-matmul
    windows (out[f + 1-kw] += p[f]) per PSUM bank.
  - Reflect columns fold into weights (exact): out[.,0]'s reflect term
    x[.,1]*w_k0[.,0] and its kw=2 term share the x factor -> host adds
    w_k0[:,0] into w_k2[:,0] (symmetric at col 127).
  - DVE does only the 9 tap products: 3 mega tensor_mul per phase (one per
    kw, all kh at once, 6144 els at 2x) -> ~169us busy = the bottleneck.
  - PE tap-sum ~123us, ACT evac f32->f16 ~30us, DMA ~76us: all hidden.
  - Output stored f16 to a packed layout, host unpacks + casts to f32.

Partition mapping: p = q*32 + wc (q = row-quarter of the chunk, wc = weight
channel). Free dims = (g, row, col), channel c = g*32 + wc.
"""

import numpy as np

import concourse.tile as tile
from concourse import bacc, mybir
from concourse.ap import AP
from concourse.bass_utils import run_bass_kernel_spmd

# Problem constants (hardcoded per contract).
N, C, H, W = 8, 256, 128, 128
WC, KK = 32, 9
G = C // WC  # 8 channel groups share one weight channel
NCORES = 8

R = 32            # rows per chunk
NCHUNK = H // R   # 4
Q = R // 4        # 8 rows handled per partition (one quarter of a chunk)
XROWS = Q + 2     # rows in the x tiles (1-row halo on each side)

FP32 = mybir.dt.float32
F16 = mybir.dt.float16

HW_ = H * W
QW = Q * W
XSZ = G * XROWS * W      # 10240 x elements per partition per chunk
WSZ = KK * QW            # 9216 w elements per partition per chunk
OSZ = 2 * QW             # 2048 out elements per partition per phase

_compiled = None


def _dram_ap(t, offset, dims):
    """AP over a DRAM tensor with explicit [stride, count] dims (elements)."""
    return AP(tensor=t.ap().tensor, offset=int(offset), ap=[[int(s), int(c)] for s, c in dims])


def _sb_ap(base, offset, dims):
    """AP over an SBUF tile: keep its partition dim, custom free dims."""
    return AP(
        tensor=base.tensor,
        offset=base.offset + int(offset),
        ap=[list(base.ap[0])] + [[int(s), int(c)] for s, c in dims],
    )


def build(
    reps: int = 1,
    do_dma: bool = True,
    do_compute: bool = True,
    do_store: bool | None = None,
    do_pe: bool | None = None,
):
    do_load = do_dma
    do_store = (do_dma if do_store is None else do_store)
    do_dve = do_compute
    do_pe = (do_compute if do_pe is None else do_pe) and do_dve
    do_store = do_store and do_pe  # stores read osb, written by evac
    nc = bacc.Bacc("TRN2", target_bir_lowering=False, debug=False, num_devices=1)

    x_t = nc.dram_tensor("xp", [NCHUNK, 128, XSZ], F16, kind="ExternalInput")
    w_t = nc.dram_tensor("wp", [NCHUNK, 128, WSZ], F16, kind="ExternalInput")
    id_t = nc.dram_tensor("ident", [128, 128], F16, kind="ExternalInput")
    o_t = nc.dram_tensor("outp", [NCHUNK, 4, 128, OSZ], F16, kind="ExternalOutput")

    with tile.TileContext(nc) as tc:
        with (
            tc.tile_pool(name="const", bufs=1) as const_pool,
            tc.tile_pool(name="xe", bufs=2) as xe_pool,
            tc.tile_pool(name="wp", bufs=2) as w_pool,
            tc.tile_pool(name="prod", bufs=2) as prod_pool,
            tc.tile_pool(name="osb", bufs=2) as out_pool,
            tc.tile_pool(name="ps", bufs=2, space="PSUM") as psum_pool,
        ):
            ident = const_pool.tile([128, 128], F16)
            nc.sync.dma_start(ident[:], id_t.ap())

            def load_chunk(ch):
                xe = xe_pool.tile([128, G, XROWS, W], F16, tag="xe")
                wt = w_pool.tile([128, KK, Q, W], F16, tag="wt")
                if do_load:
                    src = _dram_ap(x_t, ch * 128 * XSZ, [(XSZ, 128), (1, XSZ)])
                    nc.sync.dma_start(
                        xe[:].rearrange("p a b c -> p (a b c)"), src
                    )
                    src = _dram_ap(w_t, ch * 128 * WSZ, [(WSZ, 128), (1, WSZ)])
                    nc.sync.dma_start(
                        wt[:].rearrange("p a b c -> p (a b c)"), src
                    )
                return xe, wt

            def run_chunk(ch, tiles):
                xe, wt = tiles
                for hf in range(2):  # half-chunks: phases (2hf, 2hf+1)
                    # kw0/kw2 products at half-chunk size (4 g in one op) to
                    # amortize the ~330ns fixed DVE op cost; kw1 per phase so
                    # PE's start matmuls unblock early.
                    pkw0 = prod_pool.tile([128, 3, 4, Q, W], F16, tag="pkw0")
                    pkw2 = prod_pool.tile([128, 3, 4, Q, W], F16, tag="pkw2")
                    p1ph = []
                    if do_dve:
                        for ph in (2 * hf, 2 * hf + 1):
                            pkw1 = prod_pool.tile(
                                [128, 3, 2, Q, W], F16, tag="pkw1"
                            )
                            p1ph.append(pkw1)
                            xin = _sb_ap(
                                xe[:],
                                2 * ph * XROWS * W,
                                [(W, 3), (XROWS * W, 2), (W, Q), (1, W)],
                            )
                            win = _sb_ap(
                                wt[:],
                                1 * QW,
                                [(3 * QW, 3), (0, 2), (W, Q), (1, W)],
                            )
                            nc.vector.tensor_mul(pkw1[:], xin, win)
                        for kw, dst_t in ((0, pkw0), (2, pkw2)):
                            xin = _sb_ap(
                                xe[:],
                                4 * hf * XROWS * W,
                                [(W, 3), (XROWS * W, 4), (W, Q), (1, W)],
                            )
                            win = _sb_ap(
                                wt[:],
                                kw * QW,
                                [(3 * QW, 3), (0, 4), (W, Q), (1, W)],
                            )
                            nc.vector.tensor_mul(dst_t[:], xin, win)

                    for phi in range(2):
                        ph = 2 * hf + phi
                        pst = psum_pool.tile([128, 2048], FP32)
                        if do_pe:
                            # PE tap-sum: per PSUM bank, windowed identity
                            # matmuls: out[f + (1-kw)] += p[f]
                            for kw, khi in (
                                (1, 0), (1, 1), (1, 2),
                                (0, 0), (0, 1), (0, 2),
                                (2, 0), (2, 1), (2, 2),
                            ):
                                s = 1 - kw
                                if kw == 1:
                                    pflat = p1ph[phi][:, khi].rearrange(
                                        "p g r c -> p (g r c)"
                                    )
                                else:
                                    src_t = pkw0 if kw == 0 else pkw2
                                    pflat = src_t[
                                        :, khi, 2 * phi : 2 * phi + 2
                                    ].rearrange("p g r c -> p (g r c)")
                                for b in range(4):
                                    j0 = max(512 * b, s) if s > 0 else 512 * b
                                    j1 = min(512 * b + 512, 2048 + min(s, 0))
                                    nc.tensor.matmul(
                                        pst[:, j0:j1],
                                        ident[:],
                                        pflat[:, j0 - s : j1 - s],
                                        start=(kw, khi) == (1, 0),
                                        stop=(kw, khi) == (2, 2),
                                    )
                        osb = out_pool.tile([128, 2048], F16)
                        if do_pe:
                            nc.scalar.copy(osb[:], pst[:])
                        if do_store:
                            dst = _dram_ap(
                                o_t,
                                (ch * 4 + ph) * 128 * OSZ,
                                [(OSZ, 128), (1, OSZ)],
                            )
                            nc.sync.dma_start(dst, osb[:])

            def emit_body():
                # software-pipelined: prefetch chunk ch+1 before computing ch
                tiles = load_chunk(0)
                for ch in range(NCHUNK):
                    nxt = load_chunk(ch + 1) if ch + 1 < NCHUNK else None
                    run_chunk(ch, tiles)
                    tiles = nxt

            if reps == 1:
                emit_body()
            else:  # timing builds: repeat the whole kernel on-device
                with tc.For_i(
                    0, reps, 1,
                    hint_engines=(mybir.EngineType.PE, mybir.EngineType.DVE),
                ):
                    emit_body()

    nc.compile()
    return nc


def _get_compiled():
    global _compiled
    if _compiled is None:
        _compiled = build()
    return _compiled


def make_core_inputs(x_i: np.ndarray, w_i: np.ndarray) -> dict:
    """Host-side packing for one sample (layout + exact weight preprocessing).

    x pack: xp[ch, p=(q,wc), (g, t, c)] = x[g*32+wc, ch*32+q*8-1+t, c] with
    row-reflect at the image edges -- each partition's chunk tile is one
    contiguous run.

    w pack: wp[ch, p=(q,wc), (k, r, c)] = w'[wc, k, flat (rs+r)*W + c + 1-kw]
    where rs = ch*32+q*8 and w' has the reflect-column folds applied
    (w_k2[:,0] += w_k0[:,0]; w_k0[:,127] += w_k2[:,127]) and the shift-garbage
    source columns zeroed (w_k0[:,:,0] = 0, w_k2[:,:,127] = 0, and the two
    plane-edge slots read 0 via padding).
    """
    xv = np.asarray(x_i, dtype=np.float16).reshape(G, WC, H, W)

    w32 = np.array(w_i, dtype=np.float32)  # [WC, KK, H, W]
    w32[:, 2::3, :, 0] += w32[:, 0::3, :, 0]
    w32[:, 0::3, :, 127] += w32[:, 2::3, :, 127]
    w32[:, 0::3, :, 0] = 0.0
    w32[:, 2::3, :, 127] = 0.0
    wflat = np.zeros((WC, KK, HW_ + 2), dtype=np.float16)
    wflat[:, :, 1 : 1 + HW_] = w32.reshape(WC, KK, HW_).astype(np.float16)

    xp = np.empty((NCHUNK, 4, WC, G, XROWS, W), dtype=np.float16)
    wp = np.empty((NCHUNK, 4, WC, KK, Q, W), dtype=np.float16)
    for ch in range(NCHUNK):
        for q in range(4):
            rs = ch * R + q * Q
            rows = np.arange(rs - 1, rs + Q + 1)
            rows[rows == -1] = 1
            rows[rows == H] = H - 2
            xp[ch, q] = xv[:, :, rows, :].transpose(1, 0, 2, 3)
            for k in range(KK):
                off = rs * W + (1 - k % 3) + 1
                wp[ch, q, :, k] = wflat[:, k, off : off + QW].reshape(WC, Q, W)
    return {
        "xp": xp.reshape(NCHUNK, 128, XSZ),
        "wp": wp.reshape(NCHUNK, 128, WSZ),
        "ident": np.eye(128, dtype=np.float16),
    }


def unpack_output(outp: np.ndarray) -> np.ndarray:
    """outp [NCHUNK, 4ph, 128p, OSZ] f16 -> out [C, H, W] f32."""
    o = outp.reshape(NCHUNK, 4, 4, WC, 2, Q, W).astype(np.float32)
    # indices: [ch, ph, q, wc, g', r, c] -> channel (2ph+g')*32+wc, row ch*32+q*8+r
    o = o.transpose(1, 4, 3, 0, 2, 5, 6)  # [ph, g', wc, ch, q, r, c]
    return np.ascontiguousarray(o.reshape(C, H, W))


def kernel(x: np.ndarray, weight: np.ndarray) -> np.ndarray:
    nc = _get_compiled()
    in_maps = [make_core_inputs(x[i], weight[i]) for i in range(NCORES)]
    res = run_bass_kernel_spmd(nc, in_maps, core_ids=list(range(NCORES)))
    return np.stack(
        [unpack_output(res.results[i]["outp"]) for i in range(NCORES)], axis=0
    )


# revision 24
# speedup vs baseline: 1.1787x; 1.0153x over previous
"""Trainium2 Bass kernel for per-pixel dynamic-weight 3x3 aggregation.

Computation (per sample):
    out[c, h, w] = sum_{kh,kw} xpad[c, h+kh, w+kw] * weight[c % WC, kh*3+kw, h, w]
with reflect padding (pad=1) of x.

Sharding: data-parallel over batch N=8 -> one sample per NeuronCore (8 cores).

v3 design:
  - f16 end-to-end: host casts x/w to f16 and casts the f16 output back.
  - Host pre-packs x and w into per-(chunk, partition)-contiguous layouts:
    每 partition's whole chunk tile is one contiguous DRAM run (x: 20.5KB,
    w: 18.4KB), so a chunk loads with ONE DMA of 128 big descriptors. The
    DMA path is descriptor-rate-bound (~10ns/desc), so this cuts the load
    path from ~9900 descriptors (99us) to ~1000 (~53us, now bus-bound).
    The x pack also materializes the row halo + row-reflect; the w pack
    applies the per-tap column shifts (taps read column-aligned), the
    reflect-column folds, and zeroes the shift-garbage slots. No device
    memsets or reflect DMAs remain.
  - The +-1 tap column shifts are undone at accumulation: PE identity-matmul
    windows (out[f + 1-kw] += p[f]) per PSUM bank.
  - Reflect columns fold into weights (exact): out[.,0]'s reflect term
    x[.,1]*w_k0[.,0] and its kw=2 term share the x factor -> host adds
    w_k0[:,0] into w_k2[:,0] (symmetric at col 127).
  - DVE does only the 9 tap products: 3 mega tensor_mul per phase (one per
    kw, all kh at once, 6144 els at 2x) -> ~169us busy = the bottleneck.
  - PE tap-sum ~123us, ACT evac f32->f16 ~30us, DMA ~76us: all hidden.
  - Output stored f16 to a packed layout, host unpacks + casts to f32.

Partition mapping: p = q*32 + wc (q = row-quarter of the chunk, wc = weight
channel). Free dims = (g, row, col), channel c = g*32 + wc.
"""

import numpy as np

import concourse.tile as tile
from concourse import bacc, mybir
from concourse.ap import AP
from concourse.bass_utils import run_bass_kernel_spmd

# Problem constants (hardcoded per contract).
N, C, H, W = 8, 256, 128, 128
WC, KK = 32, 9
G = C // WC  # 8 channel groups share one weight channel
NCORES = 8

R = 32            # rows per chunk
NCHUNK = H // R   # 4
Q = R // 4        # 8 rows handled per partition (one quarter of a chunk)
XROWS = Q + 2     # rows in the x tiles (1-row halo on each side)

FP32 = mybir.dt.float32
F16 = mybir.dt.float16

HW_ = H * W
QW = Q * W
XSZ = G * XROWS * W      # 10240 x elements per partition per chunk
WSZ = KK * QW            # 9216 w elements per partition per chunk
OSZ = 2 * QW             # 2048 out elements per partition per phase

_compiled = None


def _dram_ap(t, offset, dims):
    """AP over a DRAM tensor with explicit [stride, count] dims (elements)."""
    return AP(tensor=t.ap().tensor, offset=int(offset), ap=[[int(s), int(c)] for s, c in dims])


def _sb_ap(base, offset, dims):
    """AP over an SBUF tile: keep its partition dim, custom free dims."""
    return AP(
        tensor=base.tensor,
        offset=base.offset + int(offset),
        ap=[list(base.ap[0])] + [[int(s), int(c)] for s, c in dims],
    )


def build(
    reps: int = 1,
    do_dma: bool = True,
    do_compute: bool = True,
    do_store: bool | None = None,
    do_pe: bool | None = None,
):
    do_load = do_dma
    do_store = (do_dma if do_store is None else do_store)
    do_dve = do_compute
    do_pe = (do_compute if do_pe is None else do_pe) and do_dve
    do_store = do_store and do_pe  # stores read osb, written by evac
    nc = bacc.Bacc("TRN2", target_bir_lowering=False, debug=False, num_devices=1)

    x_t = nc.dram_tensor("xp", [NCHUNK, 128, XSZ], F16, kind="ExternalInput")
    w_t = nc.dram_tensor("wp", [NCHUNK, 128, WSZ], F16, kind="ExternalInput")
    id_t = nc.dram_tensor("ident", [128, 128], F16, kind="ExternalInput")
    o_t = nc.dram_tensor("outp", [NCHUNK, 4, 128, OSZ], F16, kind="ExternalOutput")

    with tile.TileContext(nc) as tc:
        with (
            tc.tile_pool(name="const", bufs=1) as const_pool,
            tc.tile_pool(name="xe", bufs=2) as xe_pool,
            tc.tile_pool(name="wp", bufs=2) as w_pool,
            tc.tile_pool(name="prod", bufs=3) as prod_pool,
            tc.tile_pool(name="osb", bufs=3) as out_pool,
            tc.tile_pool(name="ps", bufs=2, space="PSUM") as psum_pool,
        ):
            ident = const_pool.tile([128, 128], F16)
            nc.sync.dma_start(ident[:], id_t.ap())

            def load_chunk(ch):
                xe = xe_pool.tile([128, G, XROWS, W], F16, tag="xe")
                wt = w_pool.tile([128, KK, Q, W], F16, tag="wt")
                if do_load:
                    src = _dram_ap(x_t, ch * 128 * XSZ, [(XSZ, 128), (1, XSZ)])
                    nc.sync.dma_start(
                        xe[:].rearrange("p a b c -> p (a b c)"), src
                    )
                    src = _dram_ap(w_t, ch * 128 * WSZ, [(WSZ, 128), (1, WSZ)])
                    nc.sync.dma_start(
                        wt[:].rearrange("p a b c -> p (a b c)"), src
                    )
                return xe, wt

            def run_chunk(ch, tiles):
                xe, wt = tiles
                for ph in range(4):  # g-pair phases: g in {2ph, 2ph+1}
                    pkw0 = prod_pool.tile([128, 3, 2, Q, W], F16, tag="pkw0")
                    pkw1 = prod_pool.tile([128, 3, 2, Q, W], F16, tag="pkw1")
                    pkw2 = prod_pool.tile([128, 3, 2, Q, W], F16, tag="pkw2")
                    pkw = [pkw0, pkw1, pkw2]
                    if do_dve:
                        # 3 mega multiplies: all kh for one kw in one DVE op.
                        # kw=1 first: it's the start matmul of every PSUM bank.
                        for kw in (1, 0, 2):
                            xin = _sb_ap(
                                xe[:],
                                2 * ph * XROWS * W,
                                [(W, 3), (XROWS * W, 2), (W, Q), (1, W)],
                            )
                            win = _sb_ap(
                                wt[:],
                                kw * QW,
                                [(3 * QW, 3), (0, 2), (W, Q), (1, W)],
                            )
                            nc.vector.tensor_mul(pkw[kw][:], xin, win)

                    pst = psum_pool.tile([128, 2048], FP32)
                    if do_pe:
                        # PE tap-sum: per PSUM bank, windowed identity matmuls
                        # out[f + (1-kw)] += p[f]. kw-major order so PE can
                        # start on pkw1 before the kw0/kw2 megas finish.
                        for kw, khi in (
                            (1, 0), (1, 1), (1, 2),
                            (0, 0), (0, 1), (0, 2),
                            (2, 0), (2, 1), (2, 2),
                        ):
                            s = 1 - kw
                            pflat = pkw[kw][:, khi].rearrange(
                                "p g r c -> p (g r c)"
                            )
                            for b in range(4):
                                j0 = max(512 * b, s) if s > 0 else 512 * b
                                j1 = min(512 * b + 512, 2048 + min(s, 0))
                                nc.tensor.matmul(
                                    pst[:, j0:j1],
                                    ident[:],
                                    pflat[:, j0 - s : j1 - s],
                                    start=(kw, khi) == (1, 0),
                                    stop=(kw, khi) == (2, 2),
                                )
                    osb = out_pool.tile([128, 2048], F16)
                    if do_pe:
                        nc.scalar.copy(osb[:], pst[:])
                    if do_store:
                        dst = _dram_ap(
                            o_t,
                            (ch * 4 + ph) * 128 * OSZ,
                            [(OSZ, 128), (1, OSZ)],
                        )
                        nc.sync.dma_start(dst, osb[:])

            def emit_body():
                # software-pipelined: prefetch chunk ch+1 before computing ch
                tiles = load_chunk(0)
                for ch in range(NCHUNK):
                    nxt = load_chunk(ch + 1) if ch + 1 < NCHUNK else None
                    run_chunk(ch, tiles)
                    tiles = nxt

            if reps == 1:
                emit_body()
            else:  # timing builds: repeat the whole kernel on-device
                with tc.For_i(
                    0, reps, 1,
                    hint_engines=(mybir.EngineType.PE, mybir.EngineType.DVE),
                ):
                    emit_body()

    nc.compile()
    return nc


def _get_compiled():
    global _compiled
    if _compiled is None:
        _compiled = build()
    return _compiled


def make_core_inputs(x_i: np.ndarray, w_i: np.ndarray) -> dict:
    """Host-side packing for one sample (layout + exact weight preprocessing).

    x pack: xp[ch, p=(q,wc), (g, t, c)] = x[g*32+wc, ch*32+q*8-1+t, c] with
    row-reflect at the image edges -- each partition's chunk tile is one
    contiguous run.

    w pack: wp[ch, p=(q,wc), (k, r, c)] = w'[wc, k, flat (rs+r)*W + c + 1-kw]
    where rs = ch*32+q*8 and w' has the reflect-column folds applied
    (w_k2[:,0] += w_k0[:,0]; w_k0[:,127] += w_k2[:,127]) and the shift-garbage
    source columns zeroed (w_k0[:,:,0] = 0, w_k2[:,:,127] = 0, and the two
    plane-edge slots read 0 via padding).
    """
    xv = np.asarray(x_i, dtype=np.float16).reshape(G, WC, H, W)

    w32 = np.array(w_i, dtype=np.float32)  # [WC, KK, H, W]
    w32[:, 2::3, :, 0] += w32[:, 0::3, :, 0]
    w32[:, 0::3, :, 127] += w32[:, 2::3, :, 127]
    w32[:, 0::3, :, 0] = 0.0
    w32[:, 2::3, :, 127] = 0.0
    wflat = np.zeros((WC, KK, HW_ + 2), dtype=np.float16)
    wflat[:, :, 1 : 1 + HW_] = w32.reshape(WC, KK, HW_).astype(np.float16)

    xp = np.empty((NCHUNK, 4, WC, G, XROWS, W), dtype=np.float16)
    wp = np.empty((NCHUNK, 4, WC, KK, Q, W), dtype=np.float16)
    for ch in range(NCHUNK):
        for q in range(4):
            rs = ch * R + q * Q
            rows = np.arange(rs - 1, rs + Q + 1)
            rows[rows == -1] = 1
            rows[rows == H] = H - 2
            xp[ch, q] = xv[:, :, rows, :].transpose(1, 0, 2, 3)
            for k in range(KK):
                off = rs * W + (1 - k % 3) + 1
                wp[ch, q, :, k] = wflat[:, k, off : off + QW].reshape(WC, Q, W)
    return {
        "xp": xp.reshape(NCHUNK, 128, XSZ),
        "wp": wp.reshape(NCHUNK, 128, WSZ),
        "ident": np.eye(128, dtype=np.float16),
    }


def unpack_output(outp: np.ndarray) -> np.ndarray:
    """outp [NCHUNK, 4ph, 128p, OSZ] f16 -> out [C, H, W] f32."""
    o = outp.reshape(NCHUNK, 4, 4, WC, 2, Q, W).astype(np.float32)
    # indices: [ch, ph, q, wc, g', r, c] -> channel (2ph+g')*32+wc, row ch*32+q*8+r
    o = o.transpose(1, 4, 3, 0, 2, 5, 6)  # [ph, g', wc, ch, q, r, c]
    return np.ascontiguousarray(o.reshape(C, H, W))


def kernel(x: np.ndarray, weight: np.ndarray) -> np.ndarray:
    nc = _get_compiled()
    in_maps = [make_core_inputs(x[i], weight[i]) for i in range(NCORES)]
    res = run_bass_kernel_spmd(nc, in_maps, core_ids=list(range(NCORES)))
    return np.stack(
        [unpack_output(res.results[i]["outp"]) for i in range(NCORES)], axis=0
    )


# revision 26
# speedup vs baseline: 1.1787x; 1.0000x over previous
"""Trainium2 Bass kernel for per-pixel dynamic-weight 3x3 aggregation.

Computation (per sample):
    out[c, h, w] = sum_{kh,kw} xpad[c, h+kh, w+kw] * weight[c % WC, kh*3+kw, h, w]
with reflect padding (pad=1) of x.

Sharding: data-parallel over batch N=8 -> one sample per NeuronCore (8 cores).

v3 design:
  - f16 end-to-end: host casts x/w to f16 and casts the f16 output back.
  - Host pre-packs x and w into per-(chunk, partition)-contiguous layouts:
    each partition's whole chunk tile is one contiguous DRAM run (x: 20.5KB,
    w: 18.4KB), so a chunk loads with ONE DMA of 128 big descriptors. The
    DMA path is descriptor-rate-bound (~10ns/desc), so this cuts the load
    path from ~9900 descriptors (99us) to ~1000 (~53us, now bus-bound).
    The x pack also materializes the row halo + row-reflect; the w pack
    applies the per-tap column shifts (taps read column-aligned), the
    reflect-column folds, and zeroes the shift-garbage slots. No device
    memsets or reflect DMAs remain.
  - The +-1 tap column shifts are undone at accumulation: PE identity-matmul
    windows (out[f + 1-kw] += p[f]) per PSUM bank.
  - Reflect columns fold into weights (exact): out[.,0]'s reflect term
    x[.,1]*w_k0[.,0] and its kw=2 term share the x factor -> host adds
    w_k0[:,0] into w_k2[:,0] (symmetric at col 127).
  - DVE does only the 9 tap products: 3 mega tensor_mul per phase (one per
    kw, all kh at once, 6144 els at 2x = 2 el/lane/cycle, the fastest
    elementwise path on TRN2; STT/GpSimd measured slower or serializing)
    -> ~169us busy = the bottleneck and the chip-level floor for this op.
  - PE tap-sum ~123us, ACT evac f32->f16 ~30us, DMA ~76us: all hidden.
  - Output stored f16 to a packed layout, host unpacks + casts to f32.
  - Measured: ~162-182us per-iteration on-device (axon wall-clock deltas;
    the shared device drifts +-10-20% between epochs). Baseline was 208-227.

Partition mapping: p = q*32 + wc (q = row-quarter of the chunk, wc = weight
channel). Free dims = (g, row, col), channel c = g*32 + wc.
"""

import numpy as np

import concourse.tile as tile
from concourse import bacc, mybir
from concourse.ap import AP
from concourse.bass_utils import run_bass_kernel_spmd

# Problem constants (hardcoded per contract).
N, C, H, W = 8, 256, 128, 128
WC, KK = 32, 9
G = C // WC  # 8 channel groups share one weight channel
NCORES = 8

R = 32            # rows per chunk
NCHUNK = H // R   # 4
Q = R // 4        # 8 rows handled per partition (one quarter of a chunk)
XROWS = Q + 2     # rows in the x tiles (1-row halo on each side)

FP32 = mybir.dt.float32
F16 = mybir.dt.float16

HW_ = H * W
QW = Q * W
XSZ = G * XROWS * W      # 10240 x elements per partition per chunk
WSZ = KK * QW            # 9216 w elements per partition per chunk
OSZ = 2 * QW             # 2048 out elements per partition per phase

_compiled = None


def _dram_ap(t, offset, dims):
    """AP over a DRAM tensor with explicit [stride, count] dims (elements)."""
    return AP(tensor=t.ap().tensor, offset=int(offset), ap=[[int(s), int(c)] for s, c in dims])


def _sb_ap(base, offset, dims):
    """AP over an SBUF tile: keep its partition dim, custom free dims."""
    return AP(
        tensor=base.tensor,
        offset=base.offset + int(offset),
        ap=[list(base.ap[0])] + [[int(s), int(c)] for s, c in dims],
    )


def build(
    reps: int = 1,
    do_dma: bool = True,
    do_compute: bool = True,
    do_store: bool | None = None,
    do_pe: bool | None = None,
    w_on_scalar: bool = False,
):
    do_load = do_dma
    do_store = (do_dma if do_store is None else do_store)
    do_dve = do_compute
    do_pe = (do_compute if do_pe is None else do_pe) and do_dve
    do_store = do_store and do_pe  # stores read osb, written by evac
    nc = bacc.Bacc("TRN2", target_bir_lowering=False, debug=False, num_devices=1)

    x_t = nc.dram_tensor("xp", [NCHUNK, 128, XSZ], F16, kind="ExternalInput")
    w_t = nc.dram_tensor("wp", [NCHUNK, 128, WSZ], F16, kind="ExternalInput")
    id_t = nc.dram_tensor("ident", [128, 128], F16, kind="ExternalInput")
    o_t = nc.dram_tensor("outp", [NCHUNK, 4, 128, OSZ], F16, kind="ExternalOutput")

    with tile.TileContext(nc) as tc:
        with (
            tc.tile_pool(name="const", bufs=1) as const_pool,
            tc.tile_pool(name="xe", bufs=2) as xe_pool,
            tc.tile_pool(name="wp", bufs=2) as w_pool,
            tc.tile_pool(name="prod", bufs=3) as prod_pool,
            tc.tile_pool(name="osb", bufs=3) as out_pool,
            tc.tile_pool(name="ps", bufs=2, space="PSUM") as psum_pool,
        ):
            ident = const_pool.tile([128, 128], F16)
            nc.sync.dma_start(ident[:], id_t.ap())

            def load_chunk(ch):
                xe = xe_pool.tile([128, G, XROWS, W], F16, tag="xe")
                wt = w_pool.tile([128, KK, Q, W], F16, tag="wt")
                if do_load:
                    src = _dram_ap(x_t, ch * 128 * XSZ, [(XSZ, 128), (1, XSZ)])
                    nc.sync.dma_start(
                        xe[:].rearrange("p a b c -> p (a b c)"), src
                    )
                    src = _dram_ap(w_t, ch * 128 * WSZ, [(WSZ, 128), (1, WSZ)])
                    weng = nc.scalar if w_on_scalar else nc.sync
                    weng.dma_start(
                        wt[:].rearrange("p a b c -> p (a b c)"), src
                    )
                return xe, wt

            def run_chunk(ch, tiles):
                xe, wt = tiles
                for ph in range(4):  # g-pair phases: g in {2ph, 2ph+1}
                    pkw0 = prod_pool.tile([128, 3, 2, Q, W], F16, tag="pkw0")
                    pkw1 = prod_pool.tile([128, 3, 2, Q, W], F16, tag="pkw1")
                    pkw2 = prod_pool.tile([128, 3, 2, Q, W], F16, tag="pkw2")
                    pkw = [pkw0, pkw1, pkw2]
                    if do_dve:
                        # 3 mega multiplies: all kh for one kw in one DVE op.
                        # kw=1 first: it's the start matmul of every PSUM bank.
                        for kw in (1, 0, 2):
                            xin = _sb_ap(
                                xe[:],
                                2 * ph * XROWS * W,
                                [(W, 3), (XROWS * W, 2), (W, Q), (1, W)],
                            )
                            win = _sb_ap(
                                wt[:],
                                kw * QW,
                                [(3 * QW, 3), (0, 2), (W, Q), (1, W)],
                            )
                            nc.vector.tensor_mul(pkw[kw][:], xin, win)

                    pst = psum_pool.tile([128, 2048], FP32)
                    if do_pe:
                        # PE tap-sum: per PSUM bank, windowed identity matmuls
                        # out[f + (1-kw)] += p[f]. kw-major order so PE can
                        # start on pkw1 before the kw0/kw2 megas finish.
                        for kw, khi in (
                            (1, 0), (1, 1), (1, 2),
                            (0, 0), (0, 1), (0, 2),
                            (2, 0), (2, 1), (2, 2),
                        ):
                            s = 1 - kw
                            pflat = pkw[kw][:, khi].rearrange(
                                "p g r c -> p (g r c)"
                            )
                            for b in range(4):
                                j0 = max(512 * b, s) if s > 0 else 512 * b
                                j1 = min(512 * b + 512, 2048 + min(s, 0))
                                nc.tensor.matmul(
                                    pst[:, j0:j1],
                                    ident[:],
                                    pflat[:, j0 - s : j1 - s],
                                    start=(kw, khi) == (1, 0),
                                    stop=(kw, khi) == (2, 2),
                                )
                    osb = out_pool.tile([128, 2048], F16)
                    if do_pe:
                        nc.scalar.copy(osb[:], pst[:])
                    if do_store:
                        dst = _dram_ap(
                            o_t,
                            (ch * 4 + ph) * 128 * OSZ,
                            [(OSZ, 128), (1, OSZ)],
                        )
                        nc.sync.dma_start(dst, osb[:])

            def emit_body():
                # software-pipelined: prefetch chunk ch+1 before computing ch
                tiles = load_chunk(0)
                for ch in range(NCHUNK):
                    nxt = load_chunk(ch + 1) if ch + 1 < NCHUNK else None
                    run_chunk(ch, tiles)
                    tiles = nxt

            if reps == 1:
                emit_body()
            else:  # timing builds: repeat the whole kernel on-device
                with tc.For_i(
                    0, reps, 1,
                    hint_engines=(mybir.EngineType.PE, mybir.EngineType.DVE),
                ):
                    emit_body()

    nc.compile()
    return nc


def _get_compiled():
    global _compiled
    if _compiled is None:
        _compiled = build()
    return _compiled


def make_core_inputs(x_i: np.ndarray, w_i: np.ndarray) -> dict:
    """Host-side packing for one sample (layout + exact weight preprocessing).

    x pack: xp[ch, p=(q,wc), (g, t, c)] = x[g*32+wc, ch*32+q*8-1+t, c] with
    row-reflect at the image edges -- each partition's chunk tile is one
    contiguous run.

    w pack: wp[ch, p=(q,wc), (k, r, c)] = w'[wc, k, flat (rs+r)*W + c + 1-kw]
    where rs = ch*32+q*8 and w' has the reflect-column folds applied
    (w_k2[:,0] += w_k0[:,0]; w_k0[:,127] += w_k2[:,127]) and the shift-garbage
    source columns zeroed (w_k0[:,:,0] = 0, w_k2[:,:,127] = 0, and the two
    plane-edge slots read 0 via padding).
    """
    xv = np.asarray(x_i, dtype=np.float16).reshape(G, WC, H, W)

    w32 = np.array(w_i, dtype=np.float32)  # [WC, KK, H, W]
    w32[:, 2::3, :, 0] += w32[:, 0::3, :, 0]
    w32[:, 0::3, :, 127] += w32[:, 2::3, :, 127]
    w32[:, 0::3, :, 0] = 0.0
    w32[:, 2::3, :, 127] = 0.0
    wflat = np.zeros((WC, KK, HW_ + 2), dtype=np.float16)
    wflat[:, :, 1 : 1 + HW_] = w32.reshape(WC, KK, HW_).astype(np.float16)

    xp = np.empty((NCHUNK, 4, WC, G, XROWS, W), dtype=np.float16)
    wp = np.empty((NCHUNK, 4, WC, KK, Q, W), dtype=np.float16)
    for ch in range(NCHUNK):
        for q in range(4):
            rs = ch * R + q * Q
            rows = np.arange(rs - 1, rs + Q + 1)
            rows[rows == -1] = 1
            rows[rows == H] = H - 2
            xp[ch, q] = xv[:, :, rows, :].transpose(1, 0, 2, 3)
            for k in range(KK):
                off = rs * W + (1 - k % 3) + 1
                wp[ch, q, :, k] = wflat[:, k, off : off + QW].reshape(WC, Q, W)
    return {
        "xp": xp.reshape(NCHUNK, 128, XSZ),
        "wp": wp.reshape(NCHUNK, 128, WSZ),
        "ident": np.eye(128, dtype=np.float16),
    }


def unpack_output(outp: np.ndarray) -> np.ndarray:
    """outp [NCHUNK, 4ph, 128p, OSZ] f16 -> out [C, H, W] f32."""
    o = outp.reshape(NCHUNK, 4, 4, WC, 2, Q, W).astype(np.float32)
    # indices: [ch, ph, q, wc, g', r, c] -> channel (2ph+g')*32+wc, row ch*32+q*8+r
    o = o.transpose(1, 4, 3, 0, 2, 5, 6)  # [ph, g', wc, ch, q, r, c]
    return np.ascontiguousarray(o.reshape(C, H, W))


def kernel(x: np.ndarray, weight: np.ndarray) -> np.ndarray:
    nc = _get_compiled()
    in_maps = [make_core_inputs(x[i], weight[i]) for i in range(NCORES)]
    res = run_bass_kernel_spmd(nc, in_maps, core_ids=list(range(NCORES)))
    return np.stack(
        [unpack_output(res.results[i]["outp"]) for i in range(NCORES)], axis=0
    )


# revision 28
# speedup vs baseline: 1.1954x; 1.0142x over previous
"""Trainium2 Bass kernel for per-pixel dynamic-weight 3x3 aggregation.

Computation (per sample):
    out[c, h, w] = sum_{kh,kw} xpad[c, h+kh, w+kw] * weight[c % WC, kh*3+kw, h, w]
with reflect padding (pad=1) of x.

Sharding: data-parallel over batch N=8 -> one sample per NeuronCore (8 cores).

v3 design:
  - f16 end-to-end: host casts x/w to f16 and casts the f16 output back.
  - Host pre-packs x and w into per-(chunk, partition)-contiguous layouts:
    each partition's whole chunk tile is one contiguous DRAM run (x: 20.5KB,
    w: 18.4KB), so a chunk loads with ONE DMA of 128 big descriptors. The
    DMA path is descriptor-rate-bound (~10ns/desc), so this cuts the load
    path from ~9900 descriptors (99us) to ~1000 (~53us, now bus-bound).
    The x pack also materializes the row halo + row-reflect; the w pack
    applies the per-tap column shifts (taps read column-aligned), the
    reflect-column folds, and zeroes the shift-garbage slots. No device
    memsets or reflect DMAs remain.
  - The +-1 tap column shifts are undone at accumulation: PE identity-matmul
    windows (out[f + 1-kw] += p[f]) per PSUM bank.
  - Reflect columns fold into weights (exact): out[.,0]'s reflect term
    x[.,1]*w_k0[.,0] and its kw=2 term share the x factor -> host adds
    w_k0[:,0] into w_k2[:,0] (symmetric at col 127).
  - DVE does only the 9 tap products: 3 mega tensor_mul per phase (one per
    kw, all kh at once, 6144 els at 2x = 2 el/lane/cycle, the fastest
    elementwise path on TRN2; STT/GpSimd measured slower or serializing)
    -> ~169us busy = the bottleneck and the chip-level floor for this op.
  - PE tap-sum ~123us, ACT evac f32->f16 ~30us, DMA ~76us: all hidden.
  - Output stored f16 to a packed layout, host unpacks + casts to f32.
  - Measured: ~162-182us per-iteration on-device (axon wall-clock deltas;
    the shared device drifts +-10-20% between epochs). Baseline was 208-227.

Partition mapping: p = q*32 + wc (q = row-quarter of the chunk, wc = weight
channel). Free dims = (g, row, col), channel c = g*32 + wc.
"""

import numpy as np

import concourse.tile as tile
from concourse import bacc, mybir
from concourse.ap import AP
from concourse.bass_utils import run_bass_kernel_spmd

# Problem constants (hardcoded per contract).
N, C, H, W = 8, 256, 128, 128
WC, KK = 32, 9
G = C // WC  # 8 channel groups share one weight channel
NCORES = 8

R = 32            # rows per chunk
NCHUNK = H // R   # 4
Q = R // 4        # 8 rows handled per partition (one quarter of a chunk)
XROWS = Q + 2     # rows in the x tiles (1-row halo on each side)

FP32 = mybir.dt.float32
F16 = mybir.dt.float16

HW_ = H * W
QW = Q * W
XSZ = G * XROWS * W      # 10240 x elements per partition per chunk
WSZ = KK * QW            # 9216 w elements per partition per chunk
OSZ = 2 * QW             # 2048 out elements per partition per phase

_compiled = None


def _dram_ap(t, offset, dims):
    """AP over a DRAM tensor with explicit [stride, count] dims (elements)."""
    return AP(tensor=t.ap().tensor, offset=int(offset), ap=[[int(s), int(c)] for s, c in dims])


def _sb_ap(base, offset, dims):
    """AP over an SBUF tile: keep its partition dim, custom free dims."""
    return AP(
        tensor=base.tensor,
        offset=base.offset + int(offset),
        ap=[list(base.ap[0])] + [[int(s), int(c)] for s, c in dims],
    )


def build(
    reps: int = 1,
    do_dma: bool = True,
    do_compute: bool = True,
    do_store: bool | None = None,
    do_pe: bool | None = None,
    w_on_scalar: bool = False,
    staggered: bool = True,
):
    do_load = do_dma
    do_store = (do_dma if do_store is None else do_store)
    do_dve = do_compute
    do_pe = (do_compute if do_pe is None else do_pe) and do_dve
    do_store = do_store and do_pe  # stores read osb, written by evac
    nc = bacc.Bacc("TRN2", target_bir_lowering=False, debug=False, num_devices=1)

    x_t = nc.dram_tensor("xp", [NCHUNK, 128, XSZ], F16, kind="ExternalInput")
    w_t = nc.dram_tensor("wp", [NCHUNK, 128, WSZ], F16, kind="ExternalInput")
    id_t = nc.dram_tensor("ident", [128, 128], F16, kind="ExternalInput")
    o_t = nc.dram_tensor("outp", [NCHUNK, 4, 128, OSZ], F16, kind="ExternalOutput")

    with tile.TileContext(nc) as tc:
        with (
            tc.tile_pool(name="const", bufs=1) as const_pool,
            tc.tile_pool(name="xe", bufs=2) as xe_pool,
            tc.tile_pool(name="wp", bufs=2) as w_pool,
            tc.tile_pool(name="prod", bufs=3) as prod_pool,
            tc.tile_pool(name="osb", bufs=3) as out_pool,
            tc.tile_pool(name="ps", bufs=2, space="PSUM") as psum_pool,
        ):
            ident = const_pool.tile([128, 128], F16)
            nc.sync.dma_start(ident[:], id_t.ap())

            def load_chunk(ch):
                xe = xe_pool.tile([128, G, XROWS, W], F16, tag="xe")
                wt = w_pool.tile([128, KK, Q, W], F16, tag="wt")
                if do_load:
                    src = _dram_ap(x_t, ch * 128 * XSZ, [(XSZ, 128), (1, XSZ)])
                    nc.sync.dma_start(
                        xe[:].rearrange("p a b c -> p (a b c)"), src
                    )
                    src = _dram_ap(w_t, ch * 128 * WSZ, [(WSZ, 128), (1, WSZ)])
                    weng = nc.scalar if w_on_scalar else nc.sync
                    weng.dma_start(
                        wt[:].rearrange("p a b c -> p (a b c)"), src
                    )
                return xe, wt

            def run_chunk(ch, tiles):
                xe, wt = tiles
                for ph in range(4):  # g-pair phases: g in {2ph, 2ph+1}
                    pkw0 = prod_pool.tile([128, 3, 2, Q, W], F16, tag="pkw0")
                    pkw1 = prod_pool.tile([128, 3, 2, Q, W], F16, tag="pkw1")
                    pkw2 = prod_pool.tile([128, 3, 2, Q, W], F16, tag="pkw2")
                    pkw = [pkw0, pkw1, pkw2]
                    if do_dve:
                        # 3 mega multiplies: all kh for one kw in one DVE op.
                        # kw=1 first: it's the start matmul of every PSUM bank.
                        for kw in (1, 0, 2):
                            xin = _sb_ap(
                                xe[:],
                                2 * ph * XROWS * W,
                                [(W, 3), (XROWS * W, 2), (W, Q), (1, W)],
                            )
                            win = _sb_ap(
                                wt[:],
                                kw * QW,
                                [(3 * QW, 3), (0, 2), (W, Q), (1, W)],
                            )
                            nc.vector.tensor_mul(pkw[kw][:], xin, win)

                    pst = psum_pool.tile([128, 2048], FP32)
                    if do_pe:
                        # PE tap-sum: per PSUM bank, windowed identity matmuls
                        # out[f + (1-kw)] += p[f]. kw-major order so PE can
                        # start on pkw1 before the kw0/kw2 megas finish.
                        for kw, khi in (
                            (1, 0), (1, 1), (1, 2),
                            (0, 0), (0, 1), (0, 2),
                            (2, 0), (2, 1), (2, 2),
                        ):
                            s = 1 - kw
                            pflat = pkw[kw][:, khi].rearrange(
                                "p g r c -> p (g r c)"
                            )
                            for b in range(4):
                                j0 = max(512 * b, s) if s > 0 else 512 * b
                                j1 = min(512 * b + 512, 2048 + min(s, 0))
                                nc.tensor.matmul(
                                    pst[:, j0:j1],
                                    ident[:],
                                    pflat[:, j0 - s : j1 - s],
                                    start=(kw, khi) == (1, 0),
                                    stop=(kw, khi) == (2, 2),
                                )
                    osb = out_pool.tile([128, 2048], F16)
                    if do_pe:
                        nc.scalar.copy(osb[:], pst[:])
                    if do_store:
                        dst = _dram_ap(
                            o_t,
                            (ch * 4 + ph) * 128 * OSZ,
                            [(OSZ, 128), (1, OSZ)],
                        )
                        nc.sync.dma_start(dst, osb[:])

            def emit_body():
                # software-pipelined: prefetch chunk ch+1 before computing ch
                tiles = load_chunk(0)
                for ch in range(NCHUNK):
                    nxt = load_chunk(ch + 1) if ch + 1 < NCHUNK else None
                    run_chunk(ch, tiles)
                    tiles = nxt

            if reps == 1:
                emit_body()
            else:  # timing builds: repeat the whole kernel on-device
                with tc.For_i(
                    0, reps, 1,
                    hint_engines=(mybir.EngineType.PE, mybir.EngineType.DVE),
                    staggered_reset=staggered,
                ):
                    emit_body()

    nc.compile()
    return nc


def _get_compiled():
    global _compiled
    if _compiled is None:
        _compiled = build()
    return _compiled


def make_core_inputs(x_i: np.ndarray, w_i: np.ndarray) -> dict:
    """Host-side packing for one sample (layout + exact weight preprocessing).

    x pack: xp[ch, p=(q,wc), (g, t, c)] = x[g*32+wc, ch*32+q*8-1+t, c] with
    row-reflect at the image edges -- each partition's chunk tile is one
    contiguous run.

    w pack: wp[ch, p=(q,wc), (k, r, c)] = w'[wc, k, flat (rs+r)*W + c + 1-kw]
    where rs = ch*32+q*8 and w' has the reflect-column folds applied
    (w_k2[:,0] += w_k0[:,0]; w_k0[:,127] += w_k2[:,127]) and the shift-garbage
    source columns zeroed (w_k0[:,:,0] = 0, w_k2[:,:,127] = 0, and the two
    plane-edge slots read 0 via padding).
    """
    xv = np.asarray(x_i, dtype=np.float16).reshape(G, WC, H, W)

    w32 = np.array(w_i, dtype=np.float32)  # [WC, KK, H, W]
    w32[:, 2::3, :, 0] += w32[:, 0::3, :, 0]
    w32[:, 0::3, :, 127] += w32[:, 2::3, :, 127]
    w32[:, 0::3, :, 0] = 0.0
    w32[:, 2::3, :, 127] = 0.0
    wflat = np.zeros((WC, KK, HW_ + 2), dtype=np.float16)
    wflat[:, :, 1 : 1 + HW_] = w32.reshape(WC, KK, HW_).astype(np.float16)

    xp = np.empty((NCHUNK, 4, WC, G, XROWS, W), dtype=np.float16)
    wp = np.empty((NCHUNK, 4, WC, KK, Q, W), dtype=np.float16)
    for ch in range(NCHUNK):
        for q in range(4):
            rs = ch * R + q * Q
            rows = np.arange(rs - 1, rs + Q + 1)
            rows[rows == -1] = 1
            rows[rows == H] = H - 2
            xp[ch, q] = xv[:, :, rows, :].transpose(1, 0, 2, 3)
            for k in range(KK):
                off = rs * W + (1 - k % 3) + 1
                wp[ch, q, :, k] = wflat[:, k, off : off + QW].reshape(WC, Q, W)
    return {
        "xp": xp.reshape(NCHUNK, 128, XSZ),
        "wp": wp.reshape(NCHUNK, 128, WSZ),
        "ident": np.eye(128, dtype=np.float16),
    }


def unpack_output(outp: np.ndarray) -> np.ndarray:
    """outp [NCHUNK, 4ph, 128p, OSZ] f16 -> out [C, H, W] f32."""
    o = outp.reshape(NCHUNK, 4, 4, WC, 2, Q, W).astype(np.float32)
    # indices: [ch, ph, q, wc, g', r, c] -> channel (2ph+g')*32+wc, row ch*32+q*8+r
    o = o.transpose(1, 4, 3, 0, 2, 5, 6)  # [ph, g', wc, ch, q, r, c]
    return np.ascontiguousarray(o.reshape(C, H, W))


def kernel(x: np.ndarray, weight: np.ndarray) -> np.ndarray:
    nc = _get_compiled()
    in_maps = [make_core_inputs(x[i], weight[i]) for i in range(NCORES)]
    res = run_bass_kernel_spmd(nc, in_maps, core_ids=list(range(NCORES)))
    return np.stack(
        [unpack_output(res.results[i]["outp"]) for i in range(NCORES)], axis=0
    )


# revision 29
# speedup vs baseline: 1.2434x; 1.0402x over previous
"""Trainium2 Bass kernel for per-pixel dynamic-weight 3x3 aggregation.

Computation (per sample):
    out[c, h, w] = sum_{kh,kw} xpad[c, h+kh, w+kw] * weight[c % WC, kh*3+kw, h, w]
with reflect padding (pad=1) of x.

Sharding: data-parallel over batch N=8 -> one sample per NeuronCore (8 cores).

v3 design:
  - f16 end-to-end: host casts x/w to f16 and casts the f16 output back.
  - Host pre-packs x and w into per-(chunk, partition)-contiguous layouts:
    each partition's whole chunk tile is one contiguous DRAM run (x: 20.5KB,
    w: 18.4KB), so a chunk loads with ONE DMA of 128 big descriptors. The
    DMA path is descriptor-rate-bound (~10ns/desc), so this cuts the load
    path from ~9900 descriptors (99us) to ~1000 (~53us, now bus-bound).
    The x pack also materializes the row halo + row-reflect; the w pack
    applies the per-tap column shifts (taps read column-aligned), the
    reflect-column folds, and zeroes the shift-garbage slots. No device
    memsets or reflect DMAs remain.
  - The +-1 tap column shifts are undone at accumulation: PE identity-matmul
    windows (out[f + 1-kw] += p[f]) per PSUM bank.
  - Reflect columns fold into weights (exact): out[.,0]'s reflect term
    x[.,1]*w_k0[.,0] and its kw=2 term share the x factor -> host adds
    w_k0[:,0] into w_k2[:,0] (symmetric at col 127).
  - DVE does only the 9 tap products: 3 mega tensor_mul per phase (one per
    kw, all kh at once, 6144 els at 2x = 2 el/lane/cycle, the fastest
    elementwise path on TRN2; STT/GpSimd measured slower or serializing)
    -> ~169us busy = the bottleneck and the chip-level floor for this op.
  - PE tap-sum ~123us, ACT evac f32->f16 ~30us, DMA ~76us: all hidden.
  - Output stored f16 to a packed layout, host unpacks + casts to f32.
  - Measured: ~162-182us per-iteration on-device (axon wall-clock deltas;
    the shared device drifts +-10-20% between epochs). Baseline was 208-227.

Partition mapping: p = q*32 + wc (q = row-quarter of the chunk, wc = weight
channel). Free dims = (g, row, col), channel c = g*32 + wc.
"""

import numpy as np

import concourse.tile as tile
from concourse import bacc, mybir
from concourse.ap import AP
from concourse.bass_utils import run_bass_kernel_spmd

# Problem constants (hardcoded per contract).
N, C, H, W = 8, 256, 128, 128
WC, KK = 32, 9
G = C // WC  # 8 channel groups share one weight channel
NCORES = 8

R = 32            # rows per chunk
NCHUNK = H // R   # 4
Q = R // 4        # 8 rows handled per partition (one quarter of a chunk)
XROWS = Q + 2     # rows in the x tiles (1-row halo on each side)

FP32 = mybir.dt.float32
F16 = mybir.dt.float16

HW_ = H * W
QW = Q * W
XSZ = G * XROWS * W      # 10240 x elements per partition per chunk
WSZ = KK * QW            # 9216 w elements per partition per chunk
OSZ = 2 * QW             # 2048 out elements per partition per phase

_compiled = None

# packed weight-plane order: kw=1 group first (the first phase's dependency)
KORDER = (1, 4, 7, 0, 3, 6, 2, 5, 8)
KW_GROUP = {1: 0, 0: 1, 2: 2}


def _dram_ap(t, offset, dims):
    """AP over a DRAM tensor with explicit [stride, count] dims (elements)."""
    return AP(tensor=t.ap().tensor, offset=int(offset), ap=[[int(s), int(c)] for s, c in dims])


def _sb_ap(base, offset, dims):
    """AP over an SBUF tile: keep its partition dim, custom free dims."""
    return AP(
        tensor=base.tensor,
        offset=base.offset + int(offset),
        ap=[list(base.ap[0])] + [[int(s), int(c)] for s, c in dims],
    )


def build(
    reps: int = 1,
    do_dma: bool = True,
    do_compute: bool = True,
    do_store: bool | None = None,
    do_pe: bool | None = None,
    w_on_scalar: bool = False,
    staggered: bool = True,
):
    do_load = do_dma
    do_store = (do_dma if do_store is None else do_store)
    do_dve = do_compute
    do_pe = (do_compute if do_pe is None else do_pe) and do_dve
    do_store = do_store and do_pe  # stores read osb, written by evac
    nc = bacc.Bacc("TRN2", target_bir_lowering=False, debug=False, num_devices=1)

    x_t = nc.dram_tensor("xp", [NCHUNK, 128, XSZ], F16, kind="ExternalInput")
    w_t = nc.dram_tensor("wp", [NCHUNK, 128, WSZ], F16, kind="ExternalInput")
    id_t = nc.dram_tensor("ident", [128, 128], F16, kind="ExternalInput")
    o_t = nc.dram_tensor("outp", [NCHUNK, 4, 128, OSZ], F16, kind="ExternalOutput")

    with tile.TileContext(nc) as tc:
        with (
            tc.tile_pool(name="const", bufs=1) as const_pool,
            tc.tile_pool(name="xe", bufs=2) as xe_pool,
            tc.tile_pool(name="wp", bufs=2) as w_pool,
            tc.tile_pool(name="prod", bufs=3) as prod_pool,
            tc.tile_pool(name="osb", bufs=3) as out_pool,
            tc.tile_pool(name="ps", bufs=2, space="PSUM") as psum_pool,
        ):
            ident = const_pool.tile([128, 128], F16)
            nc.sync.dma_start(ident[:], id_t.ap())

            def load_chunk(ch):
                xe = xe_pool.tile([128, G, XROWS, W], F16, tag="xe")
                wt = w_pool.tile([128, KK, Q, W], F16, tag="wt")
                if do_load:
                    xhead = 2 * XROWS * W   # g0-1: all phase-0 x reads
                    whead = 3 * QW          # kw1 planes: phase-0's first mega
                    xf = xe[:].rearrange("p a b c -> p (a b c)")
                    wf = wt[:].rearrange("p a b c -> p (a b c)")
                    nc.sync.dma_start(
                        xf[:, 0:xhead],
                        _dram_ap(x_t, ch * 128 * XSZ, [(XSZ, 128), (1, xhead)]),
                    )
                    nc.sync.dma_start(
                        wf[:, 0:whead],
                        _dram_ap(w_t, ch * 128 * WSZ, [(WSZ, 128), (1, whead)]),
                    )
                    nc.sync.dma_start(
                        xf[:, xhead:],
                        _dram_ap(
                            x_t,
                            ch * 128 * XSZ + xhead,
                            [(XSZ, 128), (1, XSZ - xhead)],
                        ),
                    )
                    nc.sync.dma_start(
                        wf[:, whead:],
                        _dram_ap(
                            w_t,
                            ch * 128 * WSZ + whead,
                            [(WSZ, 128), (1, WSZ - whead)],
                        ),
                    )
                return xe, wt

            def run_chunk(ch, tiles):
                xe, wt = tiles
                for ph in range(4):  # g-pair phases: g in {2ph, 2ph+1}
                    pkw0 = prod_pool.tile([128, 3, 2, Q, W], F16, tag="pkw0")
                    pkw1 = prod_pool.tile([128, 3, 2, Q, W], F16, tag="pkw1")
                    pkw2 = prod_pool.tile([128, 3, 2, Q, W], F16, tag="pkw2")
                    pkw = [pkw0, pkw1, pkw2]
                    if do_dve:
                        # 3 mega multiplies: all kh for one kw in one DVE op.
                        # kw=1 first: it's the start matmul of every PSUM bank.
                        for kw in (1, 0, 2):
                            xin = _sb_ap(
                                xe[:],
                                2 * ph * XROWS * W,
                                [(W, 3), (XROWS * W, 2), (W, Q), (1, W)],
                            )
                            win = _sb_ap(
                                wt[:],
                                KW_GROUP[kw] * 3 * QW,
                                [(QW, 3), (0, 2), (W, Q), (1, W)],
                            )
                            nc.vector.tensor_mul(pkw[kw][:], xin, win)

                    pst = psum_pool.tile([128, 2048], FP32)
                    if do_pe:
                        # PE tap-sum: per PSUM bank, windowed identity matmuls
                        # out[f + (1-kw)] += p[f]. kw-major order so PE can
                        # start on pkw1 before the kw0/kw2 megas finish.
                        for kw, khi in (
                            (1, 0), (1, 1), (1, 2),
                            (0, 0), (0, 1), (0, 2),
                            (2, 0), (2, 1), (2, 2),
                        ):
                            s = 1 - kw
                            pflat = pkw[kw][:, khi].rearrange(
                                "p g r c -> p (g r c)"
                            )
                            for b in range(4):
                                j0 = max(512 * b, s) if s > 0 else 512 * b
                                j1 = min(512 * b + 512, 2048 + min(s, 0))
                                nc.tensor.matmul(
                                    pst[:, j0:j1],
                                    ident[:],
                                    pflat[:, j0 - s : j1 - s],
                                    start=(kw, khi) == (1, 0),
                                    stop=(kw, khi) == (2, 2),
                                )
                    osb = out_pool.tile([128, 2048], F16)
                    if do_pe:
                        nc.scalar.copy(osb[:], pst[:])
                    if do_store:
                        dst = _dram_ap(
                            o_t,
                            (ch * 4 + ph) * 128 * OSZ,
                            [(OSZ, 128), (1, OSZ)],
                        )
                        nc.sync.dma_start(dst, osb[:])

            def emit_body():
                # software-pipelined: prefetch chunk ch+1 before computing ch
                tiles = load_chunk(0)
                for ch in range(NCHUNK):
                    nxt = load_chunk(ch + 1) if ch + 1 < NCHUNK else None
                    run_chunk(ch, tiles)
                    tiles = nxt

            if reps == 1:
                emit_body()
            else:  # timing builds: repeat the whole kernel on-device
                with tc.For_i(
                    0, reps, 1,
                    hint_engines=(mybir.EngineType.PE, mybir.EngineType.DVE),
                    staggered_reset=staggered,
                ):
                    emit_body()

    nc.compile()
    return nc


def _get_compiled():
    global _compiled
    if _compiled is None:
        _compiled = build()
    return _compiled


def make_core_inputs(x_i: np.ndarray, w_i: np.ndarray) -> dict:
    """Host-side packing for one sample (layout + exact weight preprocessing).

    x pack: xp[ch, p=(q,wc), (g, t, c)] = x[g*32+wc, ch*32+q*8-1+t, c] with
    row-reflect at the image edges -- each partition's chunk tile is one
    contiguous run.

    w pack: wp[ch, p=(q,wc), (k, r, c)] = w'[wc, k, flat (rs+r)*W + c + 1-kw]
    where rs = ch*32+q*8 and w' has the reflect-column folds applied
    (w_k2[:,0] += w_k0[:,0]; w_k0[:,127] += w_k2[:,127]) and the shift-garbage
    source columns zeroed (w_k0[:,:,0] = 0, w_k2[:,:,127] = 0, and the two
    plane-edge slots read 0 via padding).
    """
    xv = np.asarray(x_i, dtype=np.float16).reshape(G, WC, H, W)

    w32 = np.array(w_i, dtype=np.float32)  # [WC, KK, H, W]
    w32[:, 2::3, :, 0] += w32[:, 0::3, :, 0]
    w32[:, 0::3, :, 127] += w32[:, 2::3, :, 127]
    w32[:, 0::3, :, 0] = 0.0
    w32[:, 2::3, :, 127] = 0.0
    wflat = np.zeros((WC, KK, HW_ + 2), dtype=np.float16)
    wflat[:, :, 1 : 1 + HW_] = w32.reshape(WC, KK, HW_).astype(np.float16)

    xp = np.empty((NCHUNK, 4, WC, G, XROWS, W), dtype=np.float16)
    wp = np.empty((NCHUNK, 4, WC, KK, Q, W), dtype=np.float16)
    for ch in range(NCHUNK):
        for q in range(4):
            rs = ch * R + q * Q
            rows = np.arange(rs - 1, rs + Q + 1)
            rows[rows == -1] = 1
            rows[rows == H] = H - 2
            xp[ch, q] = xv[:, :, rows, :].transpose(1, 0, 2, 3)
            for i, k in enumerate(KORDER):
                off = rs * W + (1 - k % 3) + 1
                wp[ch, q, :, i] = wflat[:, k, off : off + QW].reshape(WC, Q, W)
    return {
        "xp": xp.reshape(NCHUNK, 128, XSZ),
        "wp": wp.reshape(NCHUNK, 128, WSZ),
        "ident": np.eye(128, dtype=np.float16),
    }


def unpack_output(outp: np.ndarray) -> np.ndarray:
    """outp [NCHUNK, 4ph, 128p, OSZ] f16 -> out [C, H, W] f32."""
    o = outp.reshape(NCHUNK, 4, 4, WC, 2, Q, W).astype(np.float32)
    # indices: [ch, ph, q, wc, g', r, c] -> channel (2ph+g')*32+wc, row ch*32+q*8+r
    o = o.transpose(1, 4, 3, 0, 2, 5, 6)  # [ph, g', wc, ch, q, r, c]
    return np.ascontiguousarray(o.reshape(C, H, W))


def kernel(x: np.ndarray, weight: np.ndarray) -> np.ndarray:
    nc = _get_compiled()
    in_maps = [make_core_inputs(x[i], weight[i]) for i in range(NCORES)]
    res = run_bass_kernel_spmd(nc, in_maps, core_ids=list(range(NCORES)))
    return np.stack(
        [unpack_output(res.results[i]["outp"]) for i in range(NCORES)], axis=0
    )


# revision 30
# speedup vs baseline: 1.2498x; 1.0051x over previous
"""Trainium2 Bass kernel for per-pixel dynamic-weight 3x3 aggregation.

Computation (per sample):
    out[c, h, w] = sum_{kh,kw} xpad[c, h+kh, w+kw] * weight[c % WC, kh*3+kw, h, w]
with reflect padding (pad=1) of x.

Sharding: data-parallel over batch N=8 -> one sample per NeuronCore (8 cores).

v3 design:
  - f16 end-to-end: host casts x/w to f16 and casts the f16 output back.
  - Host pre-packs x and w into per-(chunk, partition)-contiguous layouts:
    each partition's whole chunk tile is one contiguous DRAM run (x: 20.5KB,
    w: 18.4KB), so a chunk loads with ONE DMA of 128 big descriptors. The
    DMA path is descriptor-rate-bound (~10ns/desc), so this cuts the load
    path from ~9900 descriptors (99us) to ~1000 (~53us, now bus-bound).
    The x pack also materializes the row halo + row-reflect; the w pack
    applies the per-tap column shifts (taps read column-aligned), the
    reflect-column folds, and zeroes the shift-garbage slots. No device
    memsets or reflect DMAs remain.
  - The +-1 tap column shifts are undone at accumulation: PE identity-matmul
    windows (out[f + 1-kw] += p[f]) per PSUM bank.
  - Reflect columns fold into weights (exact): out[.,0]'s reflect term
    x[.,1]*w_k0[.,0] and its kw=2 term share the x factor -> host adds
    w_k0[:,0] into w_k2[:,0] (symmetric at col 127).
  - DVE does only the 9 tap products: 3 mega tensor_mul per phase (one per
    kw, all kh at once, 6144 els at 2x = 2 el/lane/cycle, the fastest
    elementwise path on TRN2; STT/GpSimd measured slower or serializing)
    -> ~169us busy = the bottleneck and the chip-level floor for this op.
  - PE tap-sum ~123us, ACT evac f32->f16 ~30us, DMA ~76us: all hidden.
  - Output stored f16 to a packed layout, host unpacks + casts to f32.
  - Measured: ~162-182us per-iteration on-device (axon wall-clock deltas;
    the shared device drifts +-10-20% between epochs). Baseline was 208-227.

Partition mapping: p = q*32 + wc (q = row-quarter of the chunk, wc = weight
channel). Free dims = (g, row, col), channel c = g*32 + wc.
"""

import numpy as np

import concourse.tile as tile
from concourse import bacc, mybir
from concourse.ap import AP
from concourse.bass_utils import run_bass_kernel_spmd

# Problem constants (hardcoded per contract).
N, C, H, W = 8, 256, 128, 128
WC, KK = 32, 9
G = C // WC  # 8 channel groups share one weight channel
NCORES = 8

R = 32            # rows per chunk
NCHUNK = H // R   # 4
Q = R // 4        # 8 rows handled per partition (one quarter of a chunk)
XROWS = Q + 2     # rows in the x tiles (1-row halo on each side)

FP32 = mybir.dt.float32
F16 = mybir.dt.float16

HW_ = H * W
QW = Q * W
XSZ = G * XROWS * W      # 10240 x elements per partition per chunk
WSZ = KK * QW            # 9216 w elements per partition per chunk
OSZ = 2 * QW             # 2048 out elements per partition per phase

_compiled = None

# packed weight-plane order: kw=1 group first (the first phase's dependency)
KORDER = (1, 4, 7, 0, 3, 6, 2, 5, 8)
KW_GROUP = {1: 0, 0: 1, 2: 2}


def _dram_ap(t, offset, dims):
    """AP over a DRAM tensor with explicit [stride, count] dims (elements)."""
    return AP(tensor=t.ap().tensor, offset=int(offset), ap=[[int(s), int(c)] for s, c in dims])


def _sb_ap(base, offset, dims):
    """AP over an SBUF tile: keep its partition dim, custom free dims."""
    return AP(
        tensor=base.tensor,
        offset=base.offset + int(offset),
        ap=[list(base.ap[0])] + [[int(s), int(c)] for s, c in dims],
    )


def build(
    reps: int = 1,
    do_dma: bool = True,
    do_compute: bool = True,
    do_store: bool | None = None,
    do_pe: bool | None = None,
    w_on_scalar: bool = False,
    staggered: bool = True,
):
    do_load = do_dma
    do_store = (do_dma if do_store is None else do_store)
    do_dve = do_compute
    do_pe = (do_compute if do_pe is None else do_pe) and do_dve
    do_store = do_store and do_pe  # stores read osb, written by evac
    nc = bacc.Bacc("TRN2", target_bir_lowering=False, debug=False, num_devices=1)

    x_t = nc.dram_tensor("xp", [NCHUNK, 128, XSZ], F16, kind="ExternalInput")
    w_t = nc.dram_tensor("wp", [NCHUNK, 128, WSZ], F16, kind="ExternalInput")
    id_t = nc.dram_tensor("ident", [128, 128], F16, kind="ExternalInput")
    o_t = nc.dram_tensor("outp", [NCHUNK, 4, 128, OSZ], F16, kind="ExternalOutput")

    with tile.TileContext(nc) as tc:
        with (
            tc.tile_pool(name="const", bufs=1) as const_pool,
            tc.tile_pool(name="xe", bufs=2) as xe_pool,
            tc.tile_pool(name="wp", bufs=2) as w_pool,
            tc.tile_pool(name="prod", bufs=3) as prod_pool,
            tc.tile_pool(name="osb", bufs=3) as out_pool,
            tc.tile_pool(name="ps", bufs=2, space="PSUM") as psum_pool,
        ):
            ident = const_pool.tile([128, 128], F16)
            nc.sync.dma_start(ident[:], id_t.ap())

            def load_chunk(ch):
                xe = xe_pool.tile([128, G, XROWS, W], F16, tag="xe")
                wt = w_pool.tile([128, KK, Q, W], F16, tag="wt")
                if do_load:
                    # pieces ordered by first-consumer time: phase 0 reads
                    # x g0-1 then the kw1/kw0/kw2 plane groups; later phases
                    # read the remaining g blocks.
                    xf = xe[:].rearrange("p a b c -> p (a b c)")
                    wf = wt[:].rearrange("p a b c -> p (a b c)")
                    xg = 2 * XROWS * W  # one g-pair of x
                    pieces = [
                        (xf, x_t, XSZ, 0, xg),
                        (wf, w_t, WSZ, 0 * QW, 3 * QW),
                        (wf, w_t, WSZ, 3 * QW, 3 * QW),
                        (wf, w_t, WSZ, 6 * QW, 3 * QW),
                        (xf, x_t, XSZ, xg, xg),
                        (xf, x_t, XSZ, 2 * xg, 2 * xg),
                    ]
                    for dstf, t, sz, off, n in pieces:
                        nc.sync.dma_start(
                            dstf[:, off : off + n],
                            _dram_ap(
                                t, ch * 128 * sz + off, [(sz, 128), (1, n)]
                            ),
                        )
                return xe, wt

            def run_chunk(ch, tiles):
                xe, wt = tiles
                for ph in range(4):  # g-pair phases: g in {2ph, 2ph+1}
                    pkw0 = prod_pool.tile([128, 3, 2, Q, W], F16, tag="pkw0")
                    pkw1 = prod_pool.tile([128, 3, 2, Q, W], F16, tag="pkw1")
                    pkw2 = prod_pool.tile([128, 3, 2, Q, W], F16, tag="pkw2")
                    pkw = [pkw0, pkw1, pkw2]
                    if do_dve:
                        # 3 mega multiplies: all kh for one kw in one DVE op.
                        # kw=1 first: it's the start matmul of every PSUM bank.
                        for kw in (1, 0, 2):
                            xin = _sb_ap(
                                xe[:],
                                2 * ph * XROWS * W,
                                [(W, 3), (XROWS * W, 2), (W, Q), (1, W)],
                            )
                            win = _sb_ap(
                                wt[:],
                                KW_GROUP[kw] * 3 * QW,
                                [(QW, 3), (0, 2), (W, Q), (1, W)],
                            )
                            nc.vector.tensor_mul(pkw[kw][:], xin, win)

                    pst = psum_pool.tile([128, 2048], FP32)
                    if do_pe:
                        # PE tap-sum: per PSUM bank, windowed identity matmuls
                        # out[f + (1-kw)] += p[f]. kw-major order so PE can
                        # start on pkw1 before the kw0/kw2 megas finish.
                        for kw, khi in (
                            (1, 0), (1, 1), (1, 2),
                            (0, 0), (0, 1), (0, 2),
                            (2, 0), (2, 1), (2, 2),
                        ):
                            s = 1 - kw
                            pflat = pkw[kw][:, khi].rearrange(
                                "p g r c -> p (g r c)"
                            )
                            for b in range(4):
                                j0 = max(512 * b, s) if s > 0 else 512 * b
                                j1 = min(512 * b + 512, 2048 + min(s, 0))
                                nc.tensor.matmul(
                                    pst[:, j0:j1],
                                    ident[:],
                                    pflat[:, j0 - s : j1 - s],
                                    start=(kw, khi) == (1, 0),
                                    stop=(kw, khi) == (2, 2),
                                )
                    osb = out_pool.tile([128, 2048], F16)
                    if do_pe:
                        nc.scalar.copy(osb[:], pst[:])
                    if do_store:
                        dst = _dram_ap(
                            o_t,
                            (ch * 4 + ph) * 128 * OSZ,
                            [(OSZ, 128), (1, OSZ)],
                        )
                        nc.sync.dma_start(dst, osb[:])

            def emit_body():
                # software-pipelined: prefetch chunk ch+1 before computing ch
                tiles = load_chunk(0)
                for ch in range(NCHUNK):
                    nxt = load_chunk(ch + 1) if ch + 1 < NCHUNK else None
                    run_chunk(ch, tiles)
                    tiles = nxt

            if reps == 1:
                emit_body()
            else:  # timing builds: repeat the whole kernel on-device
                with tc.For_i(
                    0, reps, 1,
                    hint_engines=(mybir.EngineType.PE, mybir.EngineType.DVE),
                    staggered_reset=staggered,
                ):
                    emit_body()

    nc.compile()
    return nc


def _get_compiled():
    global _compiled
    if _compiled is None:
        _compiled = build()
    return _compiled


def make_core_inputs(x_i: np.ndarray, w_i: np.ndarray) -> dict:
    """Host-side packing for one sample (layout + exact weight preprocessing).

    x pack: xp[ch, p=(q,wc), (g, t, c)] = x[g*32+wc, ch*32+q*8-1+t, c] with
    row-reflect at the image edges -- each partition's chunk tile is one
    contiguous run.

    w pack: wp[ch, p=(q,wc), (k, r, c)] = w'[wc, k, flat (rs+r)*W + c + 1-kw]
    where rs = ch*32+q*8 and w' has the reflect-column folds applied
    (w_k2[:,0] += w_k0[:,0]; w_k0[:,127] += w_k2[:,127]) and the shift-garbage
    source columns zeroed (w_k0[:,:,0] = 0, w_k2[:,:,127] = 0, and the two
    plane-edge slots read 0 via padding).
    """
    xv = np.asarray(x_i, dtype=np.float16).reshape(G, WC, H, W)

    w32 = np.array(w_i, dtype=np.float32)  # [WC, KK, H, W]
    w32[:, 2::3, :, 0] += w32[:, 0::3, :, 0]
    w32[:, 0::3, :, 127] += w32[:, 2::3, :, 127]
    w32[:, 0::3, :, 0] = 0.0
    w32[:, 2::3, :, 127] = 0.0
    wflat = np.zeros((WC, KK, HW_ + 2), dtype=np.float16)
    wflat[:, :, 1 : 1 + HW_] = w32.reshape(WC, KK, HW_).astype(np.float16)

    xp = np.empty((NCHUNK, 4, WC, G, XROWS, W), dtype=np.float16)
    wp = np.empty((NCHUNK, 4, WC, KK, Q, W), dtype=np.float16)
    for ch in range(NCHUNK):
        for q in range(4):
            rs = ch * R + q * Q
            rows = np.arange(rs - 1, rs + Q + 1)
            rows[rows == -1] = 1
            rows[rows == H] = H - 2
            xp[ch, q] = xv[:, :, rows, :].transpose(1, 0, 2, 3)
            for i, k in enumerate(KORDER):
                off = rs * W + (1 - k % 3) + 1
                wp[ch, q, :, i] = wflat[:, k, off : off + QW].reshape(WC, Q, W)
    return {
        "xp": xp.reshape(NCHUNK, 128, XSZ),
        "wp": wp.reshape(NCHUNK, 128, WSZ),
        "ident": np.eye(128, dtype=np.float16),
    }


def unpack_output(outp: np.ndarray) -> np.ndarray:
    """outp [NCHUNK, 4ph, 128p, OSZ] f16 -> out [C, H, W] f32."""
    o = outp.reshape(NCHUNK, 4, 4, WC, 2, Q, W).astype(np.float32)
    # indices: [ch, ph, q, wc, g', r, c] -> channel (2ph+g')*32+wc, row ch*32+q*8+r
    o = o.transpose(1, 4, 3, 0, 2, 5, 6)  # [ph, g', wc, ch, q, r, c]
    return np.ascontiguousarray(o.reshape(C, H, W))


def kernel(x: np.ndarray, weight: np.ndarray) -> np.ndarray:
    nc = _get_compiled()
    in_maps = [make_core_inputs(x[i], weight[i]) for i in range(NCORES)]
    res = run_bass_kernel_spmd(nc, in_maps, core_ids=list(range(NCORES)))
    return np.stack(
        [unpack_output(res.results[i]["outp"]) for i in range(NCORES)], axis=0
    )
